# revision 57
# baseline (speedup 1.0000x reference)
"""Multi-head attention Trainium2 kernel, 8-core batch+head sharded. v2.

Sharding: cores 0-3 -> batch 0, cores 4-7 -> batch 1; each core computes 4
heads. Host compacts queries by q_mask and keys by v_mask, transposes/packs
inputs, sums the 4 per-core partial output projections per batch (the
row-sharded-Wo "all-reduce"), adds bo, scatters rows back.

v2 vs v1: software-pipelined emission. The exp stream (ScalarE, the
steady-state bottleneck together with PE) starts ~5us in instead of ~35us:
Q-proj(block0)+K-proj run first, then the QK->exp stream runs continuously
while V-proj / remaining Q-proj / AV / outproj are paced into the PE stream
as filler. All PSUM drains on DVE (ScalarE does exp only), fp16 outproj and
zbc (fp32r streams at half rate), single large consumption-ordered DMAs.

Self-contained: hardcodes B=2,S=2048,D=1024,H=16,HS=64,OUT=1024.
"""
import sys, types
from collections import deque

sys.path.insert(0, '/opt/trn_rl_repo')

# ---- NTFF profile hook (image's antenv lacks axon_hooks) ----
if "antenv.axon_hooks" not in sys.modules:
    _hook_mod = types.ModuleType("antenv.axon_hooks")
    _hook_mod._hook = None
    def _set_hook(h, _m=_hook_mod):
        _m._hook = h
    def _get_hook(_m=_hook_mod):
        return _m._hook
    _hook_mod.set_axon_ntff_profile_hook = _set_hook
    _hook_mod.get_axon_ntff_profile_hook = _get_hook
    sys.modules["antenv.axon_hooks"] = _hook_mod
    try:
        from trn_agent_boot.trn_boot import _ntff_profile_via_ctypes
        _set_hook(_ntff_profile_via_ctypes('/opt/axon/libaxon_pjrt.so'))
    except Exception:
        pass

import numpy as np
import ml_dtypes
import concourse.bass as bass
import concourse.tile as tile
import concourse.mybir as mybir
from concourse import bass_utils, bacc

B, S, D, H, HS, OUT = 2, 2048, 1024, 16, 64, 1024
HPC = 4          # heads per core
NCORES = 8
DT = D // 128    # 8 d-tiles
F32 = mybir.dt.float32
F16 = mybir.dt.float16
F8 = mybir.dt.float8e4
SCALE = float(1.0 / np.sqrt(HS))
KPAD_BIAS = -1e5  # exp underflows to exactly 0.0
KSHIFT = -2.0     # exp(s-2): keeps e2 within fp8e4 range (max ~448)


def _blocks(total, w):
    return [(i * w, min(w, total - i * w)) for i in range((total + w - 1) // w)]


def _chunks_first(total, first, w=512):
    """[first, w, w, ...] chunk layout: small first chunk -> earlier stream
    start (less input DMA gates the first QK->exp)."""
    first = min(first, total)
    return [(0, first)] + [(c0 + first, cl) for c0, cl in _blocks(total - first, w)]


def build_kernel(SQP, SKP):
    """One SPMD Bass program. SQP/SKP: padded (mult of 128) query/key counts."""
    SKT = SKP // 128
    QCH = _chunks_first(SQP, 256)   # query chunks (also outproj blocks)
    KCH = _blocks(SKP, 512)         # key-projection chunks
    NQB = len(QCH)
    nc = bacc.Bacc("TRN2", target_bir_lowering=False, debug=False,
                   num_devices=NCORES)

    xq_d = [nc.dram_tensor(f'xq{i}', [128, DT, cl], F16, kind='ExternalInput').ap()
            for i, (c0, cl) in enumerate(QCH)]
    xk_d = [nc.dram_tensor(f'xk{i}', [128, DT, cl], F16, kind='ExternalInput').ap()
            for i, (c0, cl) in enumerate(KCH)]
    xv_d = nc.dram_tensor('xv', [128, DT, SKP], F16, kind='ExternalInput').ap()
    wq_d = nc.dram_tensor('wq', [128, DT, 256], F16, kind='ExternalInput').ap()
    wk_d = nc.dram_tensor('wk', [128, DT, 256], F16, kind='ExternalInput').ap()
    wv_d = nc.dram_tensor('wv', [128, DT, 256], F16, kind='ExternalInput').ap()
    wo_d = nc.dram_tensor('wo', [128, 2, OUT], F16, kind='ExternalInput').ap()
    qkb_d = nc.dram_tensor('qkb', [128, 4], F32, kind='ExternalInput').ap()
    vb_d = nc.dram_tensor('vb', [1, 256], F32, kind='ExternalInput').ap()
    kbias_d = nc.dram_tensor('kbias', [128, SKT], F32, kind='ExternalInput').ap()
    outp = nc.dram_tensor('outp', [SQP, OUT], F16, kind='ExternalOutput').ap()

    with tile.TileContext(nc) as tc, \
         nc.allow_low_precision(reason="fp16 activations are within tolerance"):
        with tc.tile_pool(name="const", bufs=1) as constp, \
             tc.tile_pool(name="xbuf", bufs=1) as xbuf, \
             tc.tile_pool(name="persist", bufs=1) as persist, \
             tc.tile_pool(name="etile", bufs=22) as etile, \
             tc.tile_pool(name="obuf", bufs=3) as obuf, \
             tc.tile_pool(name="work", bufs=2) as workp, \
             tc.tile_pool(name="ps", bufs=1, space="PSUM") as ps:

            # ---- constants / inputs ----
            wq_sb = constp.tile([128, DT, 256], F16)
            wk_sb = constp.tile([128, DT, 256], F16)
            wv_sb = constp.tile([128, DT, 256], F16)
            wo_sb = constp.tile([128, 2, OUT], F16)
            qkb_sb = constp.tile([128, 4], F32)
            vb_bc = constp.tile([128, 256], F32)
            kbias_sb = constp.tile([128, SKT], F32)
            ones_k = constp.tile([128, 1], F16)   # Z row-sum lhsT
            ones_b = constp.tile([128, 64], F16)  # zinv broadcast lhsT
            xq_sb = [xbuf.tile([128, DT, cl], F16, name=f'xqs{i}')
                     for i, (c0, cl) in enumerate(QCH)]
            xk_sb = [xbuf.tile([128, DT, cl], F16, name=f'xks{i}')
                     for i, (c0, cl) in enumerate(KCH)]
            xv_sb = xbuf.tile([128, DT, SKP], F16)

            # exp table preload: warm ACT before anything else lands
            warm = constp.tile([128, 1], F32)
            nc.vector.memset(warm, 1.0)
            nc.scalar.activation(warm, warm,
                                 mybir.ActivationFunctionType.Exp)
            nc.vector.memset(ones_k, 1.0)
            nc.vector.memset(ones_b, 1.0)

            # DMA issues in consumption order (each issue ~0.6us on Sync)
            nc.sync.dma_start(out=wq_sb, in_=wq_d)
            nc.sync.dma_start(out=xq_sb[0], in_=xq_d[0])
            nc.sync.dma_start(out=wk_sb, in_=wk_d)
            nc.sync.dma_start(out=xk_sb[0], in_=xk_d[0])
            nc.sync.dma_start(out=qkb_sb, in_=qkb_d)
            nc.sync.dma_start(out=kbias_sb, in_=kbias_d)
            for i in range(1, len(KCH)):
                nc.sync.dma_start(out=xk_sb[i], in_=xk_d[i])
            nc.sync.dma_start(out=xv_sb, in_=xv_d)
            for i in range(1, NQB):
                nc.sync.dma_start(out=xq_sb[i], in_=xq_d[i])
            nc.sync.dma_start(out=wv_sb, in_=wv_d)
            nc.sync.dma_start(out=vb_bc, in_=bass.AP(
                tensor=vb_d.tensor, offset=vb_d.offset,
                ap=[[0, 128], vb_d.ap[1]]))
            nc.sync.dma_start(out=wo_sb, in_=wo_d)

            # ---- persistent activations ----
            qt_sb = persist.tile([128, 2, SQP], F16)   # Q^T, [:,pair,:]
            kt_sb = persist.tile([128, 2, SKP], F16)
            v_sb = persist.tile([128, SKT, 256], F16)  # V natural, 4 heads
            ot_sb = persist.tile([128, 2, SQP], F16)   # normalized O^T
            zinv_sb = persist.tile([128, SQP], F16)

            # ---- emission helpers ----
            def emit_qproj_av(ci):
                """Q-proj chunk on the two 'av' slots (prologue only).
                Drains on ScalarE: it is idle before the exp stream starts."""
                c0, cl = QCH[ci]
                pp = [ps.tile([128, 512], F32, tag='av', bufs=2,
                              name=f'qp{p}') for p in range(2)]
                for t in range(DT):
                    for p in range(2):
                        nc.tensor.matmul(
                            pp[p][:, :cl],
                            wq_sb[:, t, p * 128:(p + 1) * 128],
                            xq_sb[ci][:, t, :],
                            start=(t == 0), stop=(t == DT - 1))
                for p in range(2):
                    nc.vector.tensor_scalar_add(
                        qt_sb[:, p, c0:c0 + cl], pp[p][:, :cl],
                        qkb_sb[:, p:p + 1])

            def emit_qproj_gp(ci, p):
                """Q-proj chunk, one pair, on the single 'gp' slot."""
                c0, cl = QCH[ci]
                qp = ps.tile([128, 512], F32, tag='gp', name='qpg')
                for t in range(DT):
                    nc.tensor.matmul(
                        qp[:, :cl], wq_sb[:, t, p * 128:(p + 1) * 128],
                        xq_sb[ci][:, t, :],
                        start=(t == 0), stop=(t == DT - 1))
                nc.vector.tensor_scalar_add(
                    qt_sb[:, p, c0:c0 + cl], qp[:, :cl], qkb_sb[:, p:p + 1])

            def emit_kproj(ci):
                """K-proj chunk, both pairs, on one 'st' slot (2 banks)."""
                c0, cl = KCH[ci]
                kp = ps.tile([128, 2, 512], F32, tag='st', bufs=2, name='kp')
                for t in range(DT):
                    for p in range(2):
                        nc.tensor.matmul(
                            kp[:, p, :cl],
                            wk_sb[:, t, p * 128:(p + 1) * 128],
                            xk_sb[ci][:, t, :],
                            start=(t == 0), stop=(t == DT - 1))
                for p in range(2):
                    nc.vector.tensor_scalar_add(
                        kt_sb[:, p, c0:c0 + cl], kp[:, p, :cl],
                        qkb_sb[:, 2 + p:3 + p])

            def emit_vproj(skt):
                """V-proj one key tile on the 'gp' slot."""
                pv = ps.tile([128, 512], F32, tag='gp', name='pv')
                for t in range(DT):
                    nc.tensor.matmul(
                        pv[:, :256], xv_sb[:, t, skt * 128:(skt + 1) * 128],
                        wv_sb[:, t, :],
                        start=(t == 0), stop=(t == DT - 1))
                nc.vector.tensor_add(v_sb[:, skt, :], pv[:, :256], vb_bc)

            def emit_qk_exp(b, skt):
                """Score matmuls + exp for one (q-block, key-tile)."""
                bq0, bqlen = QCH[b]
                e2s = []
                for p in range(2):
                    st = ps.tile([128, 2, 512], F32, tag='st', bufs=2,
                                 name=f'st{p}')
                    for hh in range(2):
                        nc.tensor.matmul(
                            st[:, hh, :bqlen],
                            kt_sb[hh * 64:(hh + 1) * 64, p,
                                  skt * 128:(skt + 1) * 128],
                            qt_sb[hh * 64:(hh + 1) * 64, p,
                                  bq0:bq0 + bqlen],
                            start=True, stop=True)
                    e2 = etile.tile([128, 2, bqlen], F16, tag='e',
                                    name=f'e{p}')
                    nc.scalar.activation(
                        e2, st[:, :, :bqlen],
                        mybir.ActivationFunctionType.Exp,
                        bias=kbias_sb[:, skt:skt + 1], scale=SCALE)
                    e2s.append(e2)
                return e2s

            def emit_avz(b, skt, e2s, opsum, zpt):
                bq0, bqlen = QCH[b]
                for p in range(2):
                    for hh in range(2):
                        h = p * 2 + hh
                        nc.tensor.matmul(
                            opsum[p][hh * 64:(hh + 1) * 64, :bqlen],
                            v_sb[:, skt, h * 64:(h + 1) * 64],
                            e2s[p][:, hh, :bqlen],
                            start=(skt == 0), stop=(skt == SKT - 1))
                for h in range(HPC):
                    p, hh = divmod(h, 2)
                    nc.tensor.matmul(
                        zpt[32 * h:32 * h + 1, :bqlen],
                        ones_k[:, 0:1], e2s[p][:, hh, :bqlen],
                        start=(skt == 0), stop=(skt == SKT - 1),
                        tile_position=(0, 32 * h))

            def emit_finalize(b, opsum, zpt, tag='gp'):
                """Copy z out of PSUM first (frees the zp bank in ~0.5us so
                the next block's Z matmuls don't wait out the slow DVE
                reciprocal), then reciprocal -> broadcast -> normalized O^T."""
                bq0, bqlen = QCH[b]
                z_sb = workp.tile([128, 512], F32, tag='lnz', name='zsb')
                nc.scalar.copy(z_sb[:, :bqlen], zpt[:, :bqlen])
                nc.vector.reciprocal(zinv_sb[:, bq0:bq0 + bqlen],
                                     z_sb[:, :bqlen])
                for p in range(2):
                    if tag == 'st':
                        zb = ps.tile([128, 2, 512], F32, tag='st', bufs=2,
                                     name='zb')[:, 0, :]
                    else:
                        zb = ps.tile([128, 512], F32, tag='gp', name='zb')
                    if p == 0:
                        # warm-keeper: bridge the reciprocal latency with
                        # throwaway matmuls so the PE never idles long
                        # enough for HAM to re-throttle the clock.
                        for _ in range(3):
                            nc.tensor.matmul(
                                zb[0:64, :bqlen], ones_b[:, 0:64],
                                qt_sb[:, 0, bq0:bq0 + bqlen],
                                start=True, stop=True)
                    for hh in range(2):
                        h = p * 2 + hh
                        nc.tensor.matmul(
                            zb[hh * 64:(hh + 1) * 64, :bqlen],
                            ones_b[32 * h:32 * h + 1, 0:64],
                            zinv_sb[32 * h:32 * h + 1, bq0:bq0 + bqlen],
                            start=True, stop=True,
                            tile_position=(32 * h, hh * 64))
                    zb_sb = workp.tile([128, 512], F32, tag='zb', name='zbs')
                    nc.vector.tensor_copy(zb_sb[:, :bqlen], zb[:, :bqlen])
                    nc.vector.tensor_mul(
                        ot_sb[:, p, bq0:bq0 + bqlen],
                        opsum[p][:, :bqlen], zb_sb[:, :bqlen])

            ob_tiles = {}

            def emit_outproj(b, sqt, ch, tag='gp'):
                """One 512-col half of the output projection of one 128-q
                tile; DMA out after the second half."""
                bq0, bqlen = QCH[b]
                q0 = bq0 + sqt * 128
                qn = min(128, bqlen - sqt * 128)
                if tag == 'st':
                    po = ps.tile([128, 2, 512], F32, tag='st', bufs=2,
                                 name='po')[:, 0, :]
                else:
                    po = ps.tile([128, 512], F32, tag='gp', name='po')
                for kt_i in range(2):
                    nc.tensor.matmul(
                        po[:qn, :], ot_sb[:, kt_i, q0:q0 + qn],
                        wo_sb[:, kt_i, ch * 512:(ch + 1) * 512],
                        start=(kt_i == 0), stop=(kt_i == 1))
                if ch == 0:
                    ob_tiles[(b, sqt)] = obuf.tile([128, OUT], F16,
                                                   tag='ob', name='ob')
                ob = ob_tiles[(b, sqt)]
                if (sqt + ch) % 2 == 0:
                    nc.vector.tensor_copy(ob[:qn, ch * 512:(ch + 1) * 512],
                                          po[:qn, :])
                else:
                    nc.scalar.copy(ob[:qn, ch * 512:(ch + 1) * 512],
                                   po[:qn, :])
                if ch == 1:
                    nc.sync.dma_start(out=outp[q0:q0 + qn, :],
                                      in_=ob[:qn, :])
                    del ob_tiles[(b, sqt)]

            # ---- emission driver ----
            # fillers: paced PE work that must not block the QK->exp stream.
            # V chunks lead (AV depends on them); Q-proj pairs interleave.
            fillers = deque(('K', i) for i in range(1, len(KCH)))
            qs = deque()
            for i in range(1, NQB):
                qs.append(('Q', i, 0))
                qs.append(('Q', i, 1))
            vs = deque(('V', s) for s in range(SKT))
            while qs or vs:
                if vs:
                    fillers.append(vs.popleft())
                if qs:
                    fillers.append(qs.popleft())

            out_fill = deque()     # outproj half-chunks, pushed per block
            av_pending = deque()   # (b, skt, e2s) awaiting AV+Z
            v_emitted = set()
            state = {'av_b': None, 'opsum': None, 'zpt': None, 'fin': -1}

            def emit_filler(f):
                if f[0] == 'K':
                    emit_kproj(f[1])
                elif f[0] == 'Q':
                    emit_qproj_gp(f[1], f[2])
                else:
                    emit_vproj(f[1])
                    v_emitted.add(f[1])

            def finalize_block(b, tag='gp'):
                emit_finalize(b, state['opsum'], state['zpt'], tag=tag)
                state['fin'] = b
                bq0, bqlen = QCH[b]
                for sqt in range((bqlen + 127) // 128):
                    out_fill.append((b, sqt, 0))
                    out_fill.append((b, sqt, 1))

            def try_av(budget):
                done = 0
                while av_pending and done < budget:
                    b, s, e2s = av_pending[0]
                    if s not in v_emitted:
                        break
                    if state['av_b'] != b:
                        if state['av_b'] is not None:
                            finalize_block(state['av_b'])
                        state['opsum'] = [
                            ps.tile([128, 512], F32, tag='av', bufs=2,
                                    name=f'op{p}') for p in range(2)]
                        state['zpt'] = ps.tile([128, 512], F32, tag='zp',
                                               name='zpt')
                        state['av_b'] = b
                    emit_avz(b, s, e2s, state['opsum'], state['zpt'])
                    av_pending.popleft()
                    done += 1
                return done

            # prologue: Q block0 + first K chunk, then the stream starts
            emit_qproj_av(0)
            emit_kproj(0)

            for b in range(NQB):
                for skt in range(SKT):
                    try_av(2)
                    e2s = emit_qk_exp(b, skt)
                    av_pending.append((b, skt, e2s))
                    if fillers:
                        emit_filler(fillers.popleft())
                    if out_fill:
                        emit_outproj(*out_fill.popleft())
                        if not fillers and out_fill:
                            emit_outproj(*out_fill.popleft())

            # tail: drain everything
            while av_pending:
                if try_av(1) == 0:
                    # V not yet emitted (tiny SKT corner case)
                    emit_filler(fillers.popleft())
            if state['av_b'] is not None:
                finalize_block(state['av_b'], tag='st')
            while out_fill:
                emit_outproj(*out_fill.popleft(), tag='st')

    nc.compile()
    return nc


_NC_CACHE = {}


def _get_kernel(SQP, SKP):
    key = (SQP, SKP)
    if key not in _NC_CACHE:
        _NC_CACHE[key] = build_kernel(SQP, SKP)
    return _NC_CACHE[key]


def _ref_numpy(q, k, v, Wq, bq, Wk, bk, Wv, bv, Wo, bo, qm, vm):
    """Exact-reference fallback for degenerate masks (all-zero v_mask)."""
    qp = (q @ Wq + bq).reshape(S, H, HS)
    kp = (k @ Wk + bk).reshape(S, H, HS)
    vp = (v @ Wv + bv).reshape(S, H, HS)
    a = np.einsum('qhd,khd->hqk', qp, kp) / np.sqrt(HS)
    a = a - (1.0 - vm[None, None, :]) * 1e12
    a = a - a.max(-1, keepdims=True)
    e = np.exp(a)
    p = e / e.sum(-1, keepdims=True)
    o = np.einsum('hqk,khd->qhd', p, vp).reshape(S, H * HS)
    return (o @ Wo + bo) * qm[:, None]


def run(query, key, value, Wq, bq, Wk, bk, Wv, bv, Wo, bo, q_mask, v_mask,
        trace=False):
    query = np.asarray(query, np.float32)
    key = np.asarray(key, np.float32)
    value = np.asarray(value, np.float32)
    Wq, bq = np.asarray(Wq, np.float32), np.asarray(bq, np.float32)
    Wk, bk = np.asarray(Wk, np.float32), np.asarray(bk, np.float32)
    Wv, bv = np.asarray(Wv, np.float32), np.asarray(bv, np.float32)
    Wo, bo = np.asarray(Wo, np.float32), np.asarray(bo, np.float32)
    q_mask = np.asarray(q_mask)
    v_mask = np.asarray(v_mask)

    qidx = [np.nonzero(q_mask[b])[0] for b in range(B)]
    kidx = [np.nonzero(v_mask[b])[0] for b in range(B)]
    host_fallback = [len(kidx[b]) == 0 for b in range(B)]

    nq = max([128] + [len(i) for b, i in enumerate(qidx) if not host_fallback[b]])
    nk = max([128] + [len(i) for b, i in enumerate(kidx) if not host_fallback[b]])
    SQP = ((nq + 127) // 128) * 128
    SKP = ((nk + 127) // 128) * 128
    SKT = SKP // 128
    QCH = _chunks_first(SQP, 256)
    KCH = _blocks(SKP, 512)

    nc = _get_kernel(SQP, SKP)

    in_maps = []
    for c in range(NCORES):
        b, hg = c // 4, c % 4
        hc = slice(hg * HPC * HS, (hg + 1) * HPC * HS)  # this core's 256 cols
        xq = np.zeros((SQP, D), np.float32)
        xk = np.zeros((SKP, D), np.float32)
        xv = np.zeros((SKP, D), np.float32)
        if not host_fallback[b]:
            xq[:len(qidx[b])] = query[b][qidx[b]]
            xk[:len(kidx[b])] = key[b][kidx[b]]
            xv[:len(kidx[b])] = value[b][kidx[b]]
        # [128, DT, n]: partition = d-within-chunk, then d-chunk, then token
        xqT = np.ascontiguousarray(
            xq.T.reshape(DT, 128, SQP).transpose(1, 0, 2)).astype(np.float16)
        xkT = np.ascontiguousarray(
            xk.T.reshape(DT, 128, SKP).transpose(1, 0, 2)).astype(np.float16)
        xvT = np.ascontiguousarray(
            xv.T.reshape(DT, 128, SKP).transpose(1, 0, 2)).astype(np.float16)
        qkb = np.stack([bq[hc][:128], bq[hc][128:],
                        bk[hc][:128], bk[hc][128:]], axis=1)
        nkb = len(kidx[b]) if not host_fallback[b] else 0
        kbias = np.where(np.arange(SKP) < nkb, KSHIFT,
                         KPAD_BIAS).astype(np.float32)
        im = {
            'xv': np.ascontiguousarray(xvT),
            'wq': np.ascontiguousarray(Wq[:, hc].reshape(DT, 128, 256)
                                       .transpose(1, 0, 2)).astype(np.float16),
            'wk': np.ascontiguousarray(Wk[:, hc].reshape(DT, 128, 256)
                                       .transpose(1, 0, 2)).astype(np.float16),
            'wv': np.ascontiguousarray(Wv[:, hc].reshape(DT, 128, 256)
                                       .transpose(1, 0, 2)).astype(np.float16),
            'wo': np.ascontiguousarray(Wo[hc, :].reshape(2, 128, OUT)
                                       .transpose(1, 0, 2)).astype(np.float16),
            'qkb': np.ascontiguousarray(qkb),
            'vb': np.ascontiguousarray(bv[hc].reshape(1, 256)),
            'kbias': np.ascontiguousarray(kbias.reshape(SKT, 128).T),
        }
        for i, (c0, cl) in enumerate(QCH):
            im[f'xq{i}'] = np.ascontiguousarray(xqT[:, :, c0:c0 + cl])
        for i, (c0, cl) in enumerate(KCH):
            im[f'xk{i}'] = np.ascontiguousarray(xkT[:, :, c0:c0 + cl])
        in_maps.append(im)

    res = bass_utils.run_bass_kernel_spmd(
        nc, in_maps, core_ids=list(range(NCORES)), trace=trace)

    out = np.zeros((B, S, OUT), np.float32)
    for b in range(B):
        if host_fallback[b]:
            out[b] = _ref_numpy(query[b], key[b], value[b], Wq, bq, Wk, bk,
                                Wv, bv, Wo, bo,
                                q_mask[b].astype(np.float32),
                                v_mask[b].astype(np.float32))
            continue
        acc = np.zeros((SQP, OUT), np.float32)
        for c in range(4 * b, 4 * b + 4):
            acc += res.results[c]['outp'].astype(np.float32)
        nqb = len(qidx[b])
        out[b][qidx[b]] = acc[:nqb] + bo
    return out, res


def kernel(**inputs):
    out, _ = run(**inputs)
    return out
